# revision 11
# baseline (speedup 1.0000x reference)
"""Averaged Hausdorff loss on 8 Trainium2 cores.

Math: d2[i,j] = |x_i|^2 + |y_j|^2 - 2 x_i.y_j via an augmented inner product
on the PE. fp32 matmul runs at 1/4 rate on TRN2, so each fp32 value is split
into hi+lo fp16 halves (~22 effective mantissa bits) and the product expanded
into K=13 fp16 contraction dims (the xl*yl term, ~1e-6, is dropped):
  dims 0-2 : xh_k * (-2 yh_k)
  dims 3-5 : xh_k * (-2 yl_k)
  dims 6-8 : xl_k * (-2 yh_k)
  dims 9-10: |x|^2 (hi, lo) * 1
  dims 11-12: 1 * |y|^2 (hi, lo)
PSUM accumulates in fp32. sqrt is monotonic so mins are taken over d2 and
sqrt'd on the host.

Sharding: set1 rows across 8 cores (2048 rows/core vs all 16384 set2 rows).
Per (128-row block, 2048-col group): 4 matmuls fill a 4-bank PSUM tile; the
Scalar engine evacuates it to SBUF fp16 (also serving as the col-min init on
the first block); the Vector engine then does
  - row-mins: tensor_tensor_reduce on chunk pairs (elementwise min of two
    512-col chunks + free-axis min-reduce chained through rowmin_sb),
  - col-mins: one fp16 2x tensor_tensor min into the running R16 buffer.
Column partials finish with PE transposes + free-axis reduces; the host
min-combines partials across cores, sqrts, and averages.
"""

import numpy as np
from contextlib import ExitStack

import concourse.bacc as bacc
import concourse.mybir as mybir
import concourse.tile as tile
import concourse.bass_isa as bass_isa
from concourse.bass_utils import run_bass_kernel_spmd

f32 = mybir.dt.float32
f16 = mybir.dt.float16
N = 16384
M = 16384
NCORES = 8
NLOC = N // NCORES       # 2048 set1 rows per core
RB = NLOC // 128         # 16 row blocks
CHUNK = 512              # matmul free dim (one PSUM bank of f32)
GROUP = 4 * CHUNK        # 2048 cols per PSUM tile (4 banks)
NG = M // GROUP          # 8 groups
TCOLS = M // 128         # 128 transpose tiles for the column-min finish
KDIM = 13

_compiled = None


def _build():
    nc = bacc.Bacc()
    xa_d = nc.dram_tensor("xa", [KDIM, NLOC], f16, kind="ExternalInput")
    ya_d = nc.dram_tensor("ya", [KDIM, M], f16, kind="ExternalInput")
    rowmin_d = nc.dram_tensor("rowmin", [128, RB], f32, kind="ExternalOutput")
    colmin_d = nc.dram_tensor("colmin", [1, M], f16, kind="ExternalOutput")

    AX = mybir.AxisListType.X
    MIN = mybir.AluOpType.max  # chains run on negated d2

    with tile.TileContext(nc) as tc:
        with ExitStack() as ctx:
            iop = ctx.enter_context(tc.tile_pool(name="io", bufs=1))
            sbp = ctx.enter_context(tc.tile_pool(name="sb16", bufs=4))
            scrp = ctx.enter_context(tc.tile_pool(name="scr", bufs=3))
            psmm = ctx.enter_context(tc.tile_pool(name="psmm", bufs=2, space="PSUM"))

            xa = iop.tile([KDIM, NLOC], f16)
            nc.sync.dma_start(xa[:], xa_d[:])
            ya_t = []
            for g in range(NG):
                yg = iop.tile([KDIM, GROUP], f16, tag=f"ya{g}")
                nc.sync.dma_start(yg[:], ya_d[:, g * GROUP:(g + 1) * GROUP])
                ya_t.append(yg)

            R16 = iop.tile([128, M], f16)        # running col-min, d2, fp16
            rowmin_sb = iop.tile([128, RB], f32)
            rmw = iop.tile([128, RB, NG], f16)

            for b in range(RB):
                for g in range(NG):
                    ps = psmm.tile([128, GROUP], f32, tag="mm")
                    for k in range(4):
                        c = g * 4 + k
                        nc.tensor.matmul(
                            ps[:, k * CHUNK:(k + 1) * CHUNK],
                            xa[:, b * 128:(b + 1) * 128],
                            ya_t[g][:, k * CHUNK:(k + 1) * CHUNK],
                            start=True,
                            stop=True,
                        )
                    Rg = R16[:, g * GROUP:(g + 1) * GROUP]
                    if b == 0:
                        # evacuate+convert straight into R16 (col-min init)
                        nc.scalar.mul(Rg, ps[:], -1.0)
                        sb = Rg
                    else:
                        sbt = sbp.tile([128, GROUP], f16, tag="sb16")
                        nc.scalar.mul(sbt[:], ps[:], -1.0)
                        sb = sbt[:]
                        nc.vector.tensor_tensor(Rg, Rg, sb, MIN)
                    f1 = scrp.tile([128, 1024], f16, tag="scr")
                    nc.vector.tensor_tensor(f1[:], sb[:, 0:1024], sb[:, 1024:2048], MIN)
                    f2 = scrp.tile([128, 512], f16, tag="scr2")
                    nc.vector.tensor_tensor(f2[:], f1[:, 0:512], f1[:, 512:1024], MIN)
                    f3 = scrp.tile([128, 256], f16, tag="scr3")
                    nc.vector.tensor_tensor(f3[:], f2[:, 0:256], f2[:, 256:512], MIN)
                    nc.vector.tensor_reduce(
                        rmw[:, b:b + 1, g], f3[:], axis=AX, op=MIN
                    )

            for b in range(RB):
                nc.vector.tensor_reduce(
                    rowmin_sb[:, b:b + 1], rmw[:, b, :], axis=AX, op=MIN
                )

            for g in range(NG):
                cred = scrp.tile([128, GROUP], f16, tag="cred")
                nc.gpsimd.partition_all_reduce(
                    cred[:],
                    R16[:, g * GROUP:(g + 1) * GROUP],
                    channels=128,
                    reduce_op=bass_isa.ReduceOp.max,
                )
                nc.sync.dma_start(
                    colmin_d[:, g * GROUP:(g + 1) * GROUP], cred[0:1, :]
                )

            nc.sync.dma_start(rowmin_d[:], rowmin_sb[:])
    nc.finalize()
    return nc


def _split16(a32):
    """fp32 [k, n] -> (hi, lo) fp16 pair with hi+lo ~ a32 (22-bit mantissa)."""
    hi = a32.astype(np.float16)
    lo = (a32 - hi.astype(np.float32)).astype(np.float16)
    return hi, lo


def _prep_inputs(set1, set2):
    s1 = np.asarray(set1, dtype=np.float32)
    s2 = np.asarray(set2, dtype=np.float32)
    n1 = (s1.astype(np.float64) ** 2).sum(1)[None].astype(np.float32)
    n2 = (s2.astype(np.float64) ** 2).sum(1)[None].astype(np.float32)
    xh, xl = _split16(s1.T)
    yh, yl = _split16(s2.T)
    nxh, nxl = _split16(n1)
    nyh, nyl = _split16(n2)
    m2yh = (-2.0 * yh.astype(np.float32)).astype(np.float16)  # exact
    m2yl = (-2.0 * yl.astype(np.float32)).astype(np.float16)  # exact
    ones_n = np.ones((1, N), np.float16)
    ones_m = np.ones((1, M), np.float16)
    XA = np.concatenate([xh, xh, xl, nxh, nxl, ones_n, ones_n], axis=0)
    YR = np.concatenate([m2yh, m2yl, m2yh, ones_m, ones_m, nyh, nyl], axis=0)
    assert XA.shape == (KDIM, N) and YR.shape == (KDIM, M)
    return np.ascontiguousarray(XA), np.ascontiguousarray(YR)


def _run(nc, XA, YR, trace=False, **kw):
    in_maps = [
        {"xa": np.ascontiguousarray(XA[:, c * NLOC:(c + 1) * NLOC]), "ya": YR}
        for c in range(NCORES)
    ]
    return run_bass_kernel_spmd(nc, in_maps, list(range(NCORES)), trace=trace, **kw)


def _combine(res):
    rowmins, colmins = [], []
    for i in range(NCORES):
        rowmins.append(res.results[i]["rowmin"].T.ravel())
        colmins.append(res.results[i]["colmin"].ravel())
    rowmin_d2 = -np.concatenate(rowmins).astype(np.float32)
    colmin_d2 = -np.max(np.stack(colmins), axis=0).astype(np.float32)
    term1 = np.sqrt(np.maximum(rowmin_d2, 0.0)).mean()
    term2 = np.sqrt(np.maximum(colmin_d2, 0.0)).mean()
    return np.asarray(term1 + term2, dtype=np.float32)


def kernel(set1: np.ndarray, set2: np.ndarray) -> np.ndarray:
    global _compiled
    if _compiled is None:
        _compiled = _build()
    XA, YR = _prep_inputs(set1, set2)
    res = _run(_compiled, XA, YR)
    return _combine(res)


# revision 12
# speedup vs baseline: 1.0426x; 1.0426x over previous
"""Averaged Hausdorff loss on 8 Trainium2 cores.

Math: d2[i,j] = |x_i|^2 + |y_j|^2 - 2 x_i.y_j via an augmented inner product
on the PE. fp32 matmul runs at 1/4 rate on TRN2, so each fp32 value is split
into hi+lo fp16 halves (~22 effective mantissa bits) and the product expanded
into K=13 fp16 contraction dims (the xl*yl term, ~1e-6, is dropped):
  dims 0-2 : xh_k * (-2 yh_k)
  dims 3-5 : xh_k * (-2 yl_k)
  dims 6-8 : xl_k * (-2 yh_k)
  dims 9-10: |x|^2 (hi, lo) * 1
  dims 11-12: 1 * |y|^2 (hi, lo)
PSUM accumulates in fp32. sqrt is monotonic so mins are taken over d2 and
sqrt'd on the host.

Sharding: set1 rows across 8 cores (2048 rows/core vs all 16384 set2 rows).
Per (128-row block, 2048-col group): 4 matmuls fill a 4-bank PSUM tile; the
Scalar engine evacuates it to SBUF fp16 (also serving as the col-min init on
the first block); the Vector engine then does
  - row-mins: tensor_tensor_reduce on chunk pairs (elementwise min of two
    512-col chunks + free-axis min-reduce chained through rowmin_sb),
  - col-mins: one fp16 2x tensor_tensor min into the running R16 buffer.
Column partials finish with PE transposes + free-axis reduces; the host
min-combines partials across cores, sqrts, and averages.
"""

import numpy as np
from contextlib import ExitStack

import concourse.bacc as bacc
import concourse.mybir as mybir
import concourse.tile as tile
import concourse.bass_isa as bass_isa
from concourse.bass_utils import run_bass_kernel_spmd

f32 = mybir.dt.float32
f16 = mybir.dt.float16
N = 16384
M = 16384
NCORES = 8
NLOC = N // NCORES       # 2048 set1 rows per core
RB = NLOC // 128         # 16 row blocks
CHUNK = 512              # matmul free dim (one PSUM bank of f32)
GROUP = 4 * CHUNK        # 2048 cols per PSUM tile (4 banks)
NG = M // GROUP          # 8 groups
TCOLS = M // 128         # 128 transpose tiles for the column-min finish
KDIM = 13

_compiled = None


def _build():
    nc = bacc.Bacc()
    xa_d = nc.dram_tensor("xa", [KDIM, NLOC], f16, kind="ExternalInput")
    ya_d = nc.dram_tensor("ya", [KDIM, M], f16, kind="ExternalInput")
    rowmin_d = nc.dram_tensor("rowmin", [128, RB], f32, kind="ExternalOutput")
    colmin_d = nc.dram_tensor("colmin", [1, M], f16, kind="ExternalOutput")

    AX = mybir.AxisListType.X
    MIN = mybir.AluOpType.max  # chains run on negated d2

    with tile.TileContext(nc) as tc:
        with ExitStack() as ctx:
            iop = ctx.enter_context(tc.tile_pool(name="io", bufs=1))
            sbp = ctx.enter_context(tc.tile_pool(name="sb16", bufs=4))
            scrp = ctx.enter_context(tc.tile_pool(name="scr", bufs=3))
            psmm = ctx.enter_context(tc.tile_pool(name="psmm", bufs=2, space="PSUM"))

            xa = iop.tile([KDIM, NLOC], f16)
            nc.sync.dma_start(xa[:], xa_d[:])
            ya_t = []
            for g in range(NG):
                yg = iop.tile([KDIM, GROUP], f16, tag=f"ya{g}")
                nc.sync.dma_start(yg[:], ya_d[:, g * GROUP:(g + 1) * GROUP])
                ya_t.append(yg)

            R16 = iop.tile([128, M], f16)        # running col-min, d2, fp16
            rowmin_sb = iop.tile([128, RB], f32)
            rmw = iop.tile([128, RB, NG], f16)

            for g in range(NG):
                for b in range(RB):
                    ps = psmm.tile([128, GROUP], f32, tag="mm")
                    for k in range(4):
                        c = g * 4 + k
                        nc.tensor.matmul(
                            ps[:, k * CHUNK:(k + 1) * CHUNK],
                            xa[:, b * 128:(b + 1) * 128],
                            ya_t[g][:, k * CHUNK:(k + 1) * CHUNK],
                            start=True,
                            stop=True,
                        )
                    Rg = R16[:, g * GROUP:(g + 1) * GROUP]
                    if b == 0:
                        # evacuate+convert straight into R16 (col-min init)
                        nc.scalar.mul(Rg, ps[:], -1.0)
                        sb = Rg
                    else:
                        sbt = sbp.tile([128, GROUP], f16, tag="sb16")
                        nc.scalar.mul(sbt[:], ps[:], -1.0)
                        sb = sbt[:]
                        nc.vector.tensor_tensor(Rg, Rg, sb, MIN)
                    f1 = scrp.tile([128, 1024], f16, tag="scr")
                    nc.vector.tensor_tensor(f1[:], sb[:, 0:1024], sb[:, 1024:2048], MIN)
                    f2 = scrp.tile([128, 512], f16, tag="scr2")
                    nc.vector.tensor_tensor(f2[:], f1[:, 0:512], f1[:, 512:1024], MIN)
                    f3 = scrp.tile([128, 256], f16, tag="scr3")
                    nc.vector.tensor_tensor(f3[:], f2[:, 0:256], f2[:, 256:512], MIN)
                    nc.vector.tensor_reduce(
                        rmw[:, b:b + 1, g], f3[:], axis=AX, op=MIN
                    )
                cred = scrp.tile([128, GROUP], f16, tag="cred")
                nc.gpsimd.partition_all_reduce(
                    cred[:],
                    R16[:, g * GROUP:(g + 1) * GROUP],
                    channels=128,
                    reduce_op=bass_isa.ReduceOp.max,
                )
                nc.sync.dma_start(
                    colmin_d[:, g * GROUP:(g + 1) * GROUP], cred[0:1, :]
                )

            for b in range(RB):
                nc.vector.tensor_reduce(
                    rowmin_sb[:, b:b + 1], rmw[:, b, :], axis=AX, op=MIN
                )

            nc.sync.dma_start(rowmin_d[:], rowmin_sb[:])
    nc.finalize()
    return nc


def _split16(a32):
    """fp32 [k, n] -> (hi, lo) fp16 pair with hi+lo ~ a32 (22-bit mantissa)."""
    hi = a32.astype(np.float16)
    lo = (a32 - hi.astype(np.float32)).astype(np.float16)
    return hi, lo


def _prep_inputs(set1, set2):
    s1 = np.asarray(set1, dtype=np.float32)
    s2 = np.asarray(set2, dtype=np.float32)
    n1 = (s1.astype(np.float64) ** 2).sum(1)[None].astype(np.float32)
    n2 = (s2.astype(np.float64) ** 2).sum(1)[None].astype(np.float32)
    xh, xl = _split16(s1.T)
    yh, yl = _split16(s2.T)
    nxh, nxl = _split16(n1)
    nyh, nyl = _split16(n2)
    m2yh = (-2.0 * yh.astype(np.float32)).astype(np.float16)  # exact
    m2yl = (-2.0 * yl.astype(np.float32)).astype(np.float16)  # exact
    ones_n = np.ones((1, N), np.float16)
    ones_m = np.ones((1, M), np.float16)
    XA = np.concatenate([xh, xh, xl, nxh, nxl, ones_n, ones_n], axis=0)
    YR = np.concatenate([m2yh, m2yl, m2yh, ones_m, ones_m, nyh, nyl], axis=0)
    assert XA.shape == (KDIM, N) and YR.shape == (KDIM, M)
    return np.ascontiguousarray(XA), np.ascontiguousarray(YR)


def _run(nc, XA, YR, trace=False, **kw):
    in_maps = [
        {"xa": np.ascontiguousarray(XA[:, c * NLOC:(c + 1) * NLOC]), "ya": YR}
        for c in range(NCORES)
    ]
    return run_bass_kernel_spmd(nc, in_maps, list(range(NCORES)), trace=trace, **kw)


def _combine(res):
    rowmins, colmins = [], []
    for i in range(NCORES):
        rowmins.append(res.results[i]["rowmin"].T.ravel())
        colmins.append(res.results[i]["colmin"].ravel())
    rowmin_d2 = -np.concatenate(rowmins).astype(np.float32)
    colmin_d2 = -np.max(np.stack(colmins), axis=0).astype(np.float32)
    term1 = np.sqrt(np.maximum(rowmin_d2, 0.0)).mean()
    term2 = np.sqrt(np.maximum(colmin_d2, 0.0)).mean()
    return np.asarray(term1 + term2, dtype=np.float32)


def kernel(set1: np.ndarray, set2: np.ndarray) -> np.ndarray:
    global _compiled
    if _compiled is None:
        _compiled = _build()
    XA, YR = _prep_inputs(set1, set2)
    res = _run(_compiled, XA, YR)
    return _combine(res)


# revision 13
# speedup vs baseline: 1.0730x; 1.0291x over previous
"""Averaged Hausdorff loss on 8 Trainium2 cores.

Math: d2[i,j] = |x_i|^2 + |y_j|^2 - 2 x_i.y_j via an augmented inner product
on the PE. fp32 matmul runs at 1/4 rate on TRN2, so each fp32 value is split
into hi+lo fp16 halves (~22 effective mantissa bits) and the product expanded
into K=13 fp16 contraction dims (the xl*yl term, ~1e-6, is dropped):
  dims 0-2 : xh_k * (-2 yh_k)
  dims 3-5 : xh_k * (-2 yl_k)
  dims 6-8 : xl_k * (-2 yh_k)
  dims 9-10: |x|^2 (hi, lo) * 1
  dims 11-12: 1 * |y|^2 (hi, lo)
PSUM accumulates in fp32. sqrt is monotonic so mins are taken over d2 and
sqrt'd on the host.

Sharding: set1 rows across 8 cores (2048 rows/core vs all 16384 set2 rows).
Per (128-row block, 2048-col group): 4 matmuls fill a 4-bank PSUM tile; the
Scalar engine evacuates it to SBUF fp16 (also serving as the col-min init on
the first block); the Vector engine then does
  - row-mins: tensor_tensor_reduce on chunk pairs (elementwise min of two
    512-col chunks + free-axis min-reduce chained through rowmin_sb),
  - col-mins: one fp16 2x tensor_tensor min into the running R16 buffer.
Column partials finish with PE transposes + free-axis reduces; the host
min-combines partials across cores, sqrts, and averages.
"""

import numpy as np
from contextlib import ExitStack

import concourse.bacc as bacc
import concourse.mybir as mybir
import concourse.tile as tile
import concourse.bass_isa as bass_isa
from concourse.bass_utils import run_bass_kernel_spmd

f32 = mybir.dt.float32
f16 = mybir.dt.float16
N = 16384
M = 16384
NCORES = 8
NLOC = N // NCORES       # 2048 set1 rows per core
RB = NLOC // 128         # 16 row blocks
CHUNK = 512              # matmul free dim (one PSUM bank of f32)
GROUP = 4 * CHUNK        # 2048 cols per PSUM tile (4 banks)
NG = M // GROUP          # 8 groups
TCOLS = M // 128         # 128 transpose tiles for the column-min finish
KDIM = 13

_compiled = None


def _build():
    nc = bacc.Bacc()
    xa_d = nc.dram_tensor("xa", [KDIM, NLOC], f16, kind="ExternalInput")
    ya_d = nc.dram_tensor("ya", [KDIM, M], f16, kind="ExternalInput")
    rowmin_d = nc.dram_tensor("rowmin", [128, RB], f32, kind="ExternalOutput")
    colmin_d = nc.dram_tensor("colmin", [1, M], f16, kind="ExternalOutput")

    AX = mybir.AxisListType.X
    MIN = mybir.AluOpType.max  # chains run on negated d2

    with tile.TileContext(nc) as tc:
        with ExitStack() as ctx:
            iop = ctx.enter_context(tc.tile_pool(name="io", bufs=1))
            sbp = ctx.enter_context(tc.tile_pool(name="sb16", bufs=4))
            scrp = ctx.enter_context(tc.tile_pool(name="scr", bufs=3))
            psmm = ctx.enter_context(tc.tile_pool(name="psmm", bufs=2, space="PSUM"))

            xa = iop.tile([KDIM, NLOC], f16)
            nc.sync.dma_start(xa[:], xa_d[:])
            ya_t = []
            for g in range(NG):
                yg = iop.tile([KDIM, GROUP], f16, tag=f"ya{g}")
                nc.sync.dma_start(yg[:], ya_d[:, g * GROUP:(g + 1) * GROUP])
                ya_t.append(yg)

            R16 = iop.tile([128, M], f16)        # running col-min, d2, fp16
            rowmin_sb = iop.tile([128, RB], f32)
            rmw = iop.tile([128, RB, NG // 2], f16)
            f1stash = iop.tile([128, RB, 1024], f16)

            for g in range(NG):
                for b in range(RB):
                    ps = psmm.tile([128, GROUP], f32, tag="mm")
                    for k in range(4):
                        c = g * 4 + k
                        nc.tensor.matmul(
                            ps[:, k * CHUNK:(k + 1) * CHUNK],
                            xa[:, b * 128:(b + 1) * 128],
                            ya_t[g][:, k * CHUNK:(k + 1) * CHUNK],
                            start=True,
                            stop=True,
                        )
                    Rg = R16[:, g * GROUP:(g + 1) * GROUP]
                    if b == 0:
                        # evacuate+convert straight into R16 (col-min init)
                        nc.scalar.mul(Rg, ps[:], -1.0)
                        sb = Rg
                    else:
                        sbt = sbp.tile([128, GROUP], f16, tag="sb16")
                        nc.scalar.mul(sbt[:], ps[:], -1.0)
                        sb = sbt[:]
                        nc.vector.tensor_tensor(Rg, Rg, sb, MIN)
                    if g % 2 == 0:
                        # stash this group's 1024-wide fold; merged next group
                        nc.vector.tensor_tensor(
                            f1stash[:, b, :], sb[:, 0:1024], sb[:, 1024:2048], MIN
                        )
                    else:
                        f1 = scrp.tile([128, 1024], f16, tag="scr")
                        nc.vector.tensor_tensor(f1[:], sb[:, 0:1024], sb[:, 1024:2048], MIN)
                        f2 = scrp.tile([128, 1024], f16, tag="scr2")
                        nc.vector.tensor_tensor(f2[:], f1[:], f1stash[:, b, :], MIN)
                        f3 = scrp.tile([128, 512], f16, tag="scr3")
                        nc.vector.tensor_tensor(f3[:], f2[:, 0:512], f2[:, 512:1024], MIN)
                        f4 = scrp.tile([128, 256], f16, tag="scr4")
                        nc.vector.tensor_tensor(f4[:], f3[:, 0:256], f3[:, 256:512], MIN)
                        nc.vector.tensor_reduce(
                            rmw[:, b:b + 1, g // 2], f4[:], axis=AX, op=MIN
                        )
                cred = scrp.tile([128, GROUP], f16, tag="cred")
                nc.gpsimd.partition_all_reduce(
                    cred[:],
                    R16[:, g * GROUP:(g + 1) * GROUP],
                    channels=128,
                    reduce_op=bass_isa.ReduceOp.max,
                )
                nc.sync.dma_start(
                    colmin_d[:, g * GROUP:(g + 1) * GROUP], cred[0:1, :]
                )

            for b in range(RB):
                nc.vector.tensor_reduce(
                    rowmin_sb[:, b:b + 1], rmw[:, b, :], axis=AX, op=MIN
                )

            nc.sync.dma_start(rowmin_d[:], rowmin_sb[:])
    nc.finalize()
    return nc


def _split16(a32):
    """fp32 [k, n] -> (hi, lo) fp16 pair with hi+lo ~ a32 (22-bit mantissa)."""
    hi = a32.astype(np.float16)
    lo = (a32 - hi.astype(np.float32)).astype(np.float16)
    return hi, lo


def _prep_inputs(set1, set2):
    s1 = np.asarray(set1, dtype=np.float32)
    s2 = np.asarray(set2, dtype=np.float32)
    n1 = (s1.astype(np.float64) ** 2).sum(1)[None].astype(np.float32)
    n2 = (s2.astype(np.float64) ** 2).sum(1)[None].astype(np.float32)
    xh, xl = _split16(s1.T)
    yh, yl = _split16(s2.T)
    nxh, nxl = _split16(n1)
    nyh, nyl = _split16(n2)
    m2yh = (-2.0 * yh.astype(np.float32)).astype(np.float16)  # exact
    m2yl = (-2.0 * yl.astype(np.float32)).astype(np.float16)  # exact
    ones_n = np.ones((1, N), np.float16)
    ones_m = np.ones((1, M), np.float16)
    XA = np.concatenate([xh, xh, xl, nxh, nxl, ones_n, ones_n], axis=0)
    YR = np.concatenate([m2yh, m2yl, m2yh, ones_m, ones_m, nyh, nyl], axis=0)
    assert XA.shape == (KDIM, N) and YR.shape == (KDIM, M)
    return np.ascontiguousarray(XA), np.ascontiguousarray(YR)


def _run(nc, XA, YR, trace=False, **kw):
    in_maps = [
        {"xa": np.ascontiguousarray(XA[:, c * NLOC:(c + 1) * NLOC]), "ya": YR}
        for c in range(NCORES)
    ]
    return run_bass_kernel_spmd(nc, in_maps, list(range(NCORES)), trace=trace, **kw)


def _combine(res):
    rowmins, colmins = [], []
    for i in range(NCORES):
        rowmins.append(res.results[i]["rowmin"].T.ravel())
        colmins.append(res.results[i]["colmin"].ravel())
    rowmin_d2 = -np.concatenate(rowmins).astype(np.float32)
    colmin_d2 = -np.max(np.stack(colmins), axis=0).astype(np.float32)
    term1 = np.sqrt(np.maximum(rowmin_d2, 0.0)).mean()
    term2 = np.sqrt(np.maximum(colmin_d2, 0.0)).mean()
    return np.asarray(term1 + term2, dtype=np.float32)


def kernel(set1: np.ndarray, set2: np.ndarray) -> np.ndarray:
    global _compiled
    if _compiled is None:
        _compiled = _build()
    XA, YR = _prep_inputs(set1, set2)
    res = _run(_compiled, XA, YR)
    return _combine(res)


# revision 14
# speedup vs baseline: 1.0945x; 1.0200x over previous
"""Averaged Hausdorff loss on 8 Trainium2 cores.

Math: d2[i,j] = |x_i|^2 + |y_j|^2 - 2 x_i.y_j via an augmented inner product
on the PE. fp32 matmul runs at 1/4 rate on TRN2, so each fp32 value is split
into hi+lo fp16 halves (~22 effective mantissa bits) and the product expanded
into K=13 fp16 contraction dims (the xl*yl term, ~1e-6, is dropped):
  dims 0-2 : xh_k * (-2 yh_k)
  dims 3-5 : xh_k * (-2 yl_k)
  dims 6-8 : xl_k * (-2 yh_k)
  dims 9-10: |x|^2 (hi, lo) * 1
  dims 11-12: 1 * |y|^2 (hi, lo)
PSUM accumulates in fp32. sqrt is monotonic so mins are taken over d2 and
sqrt'd on the host.

Sharding: set1 rows across 8 cores (2048 rows/core vs all 16384 set2 rows).
Per (128-row block, 2048-col group): 4 matmuls fill a 4-bank PSUM tile; the
Scalar engine evacuates it to SBUF fp16 (also serving as the col-min init on
the first block); the Vector engine then does
  - row-mins: tensor_tensor_reduce on chunk pairs (elementwise min of two
    512-col chunks + free-axis min-reduce chained through rowmin_sb),
  - col-mins: one fp16 2x tensor_tensor min into the running R16 buffer.
Column partials finish with PE transposes + free-axis reduces; the host
min-combines partials across cores, sqrts, and averages.
"""

import numpy as np
from contextlib import ExitStack

import concourse.bacc as bacc
import concourse.mybir as mybir
import concourse.tile as tile
import concourse.bass_isa as bass_isa
from concourse.bass_utils import run_bass_kernel_spmd

f32 = mybir.dt.float32
f16 = mybir.dt.float16
N = 16384
M = 16384
NCORES = 8
NLOC = N // NCORES       # 2048 set1 rows per core
RB = NLOC // 128         # 16 row blocks
CHUNK = 512              # matmul free dim (one PSUM bank of f32)
GROUP = 4 * CHUNK        # 2048 cols per PSUM tile (4 banks)
NG = M // GROUP          # 8 groups
TCOLS = M // 128         # 128 transpose tiles for the column-min finish
KDIM = 13

_compiled = None


def _build():
    nc = bacc.Bacc()
    xa_d = nc.dram_tensor("xa", [KDIM, NLOC], f16, kind="ExternalInput")
    ya_d = nc.dram_tensor("ya", [KDIM, M], f16, kind="ExternalInput")
    rowmin_d = nc.dram_tensor("rowmin", [128, RB], f32, kind="ExternalOutput")
    colmin_d = nc.dram_tensor("colmin", [1, M], f16, kind="ExternalOutput")

    AX = mybir.AxisListType.X
    MIN = mybir.AluOpType.max  # chains run on negated d2

    with tile.TileContext(nc) as tc:
        with ExitStack() as ctx:
            iop = ctx.enter_context(tc.tile_pool(name="io", bufs=1))
            sbp = ctx.enter_context(tc.tile_pool(name="sb16", bufs=4))
            scrp = ctx.enter_context(tc.tile_pool(name="scr", bufs=2))
            psmm = ctx.enter_context(tc.tile_pool(name="psmm", bufs=2, space="PSUM"))

            xa = iop.tile([KDIM, NLOC], f16)
            nc.sync.dma_start(xa[:], xa_d[:])
            ya_t = []
            for g in range(NG):
                yg = iop.tile([KDIM, GROUP], f16, tag=f"ya{g}")
                nc.sync.dma_start(yg[:], ya_d[:, g * GROUP:(g + 1) * GROUP])
                ya_t.append(yg)

            R16 = iop.tile([128, M], f16)        # running col-min, d2, fp16
            rowmin_sb = iop.tile([128, RB], f32)
            rmw = iop.tile([128, RB, NG // 4], f16)
            stashA = iop.tile([128, RB, 1024], f16)
            stashB = iop.tile([128, RB, 1024], f16)

            for g in range(NG):
                for b in range(RB):
                    ps = psmm.tile([128, GROUP], f32, tag="mm")
                    for k in range(4):
                        c = g * 4 + k
                        nc.tensor.matmul(
                            ps[:, k * CHUNK:(k + 1) * CHUNK],
                            xa[:, b * 128:(b + 1) * 128],
                            ya_t[g][:, k * CHUNK:(k + 1) * CHUNK],
                            start=True,
                            stop=True,
                        )
                    Rg = R16[:, g * GROUP:(g + 1) * GROUP]
                    if b == 0:
                        # evacuate+convert straight into R16 (col-min init)
                        nc.scalar.mul(Rg, ps[:], -1.0)
                        sb = Rg
                    else:
                        sbt = sbp.tile([128, GROUP], f16, tag="sb16")
                        nc.scalar.mul(sbt[:], ps[:], -1.0)
                        sb = sbt[:]
                        nc.vector.tensor_tensor(Rg, Rg, sb, MIN)
                    if g % 2 == 0:
                        # stash this group's 1024-wide fold (A on quads 0, B holds 0+1)
                        nc.vector.tensor_tensor(
                            stashA[:, b, :], sb[:, 0:1024], sb[:, 1024:2048], MIN
                        )
                    elif g % 4 == 1:
                        f1 = scrp.tile([128, 1024], f16, tag="scr")
                        nc.vector.tensor_tensor(f1[:], sb[:, 0:1024], sb[:, 1024:2048], MIN)
                        nc.vector.tensor_tensor(
                            stashB[:, b, :], f1[:], stashA[:, b, :], MIN
                        )
                    else:
                        f1 = scrp.tile([128, 1024], f16, tag="scr")
                        nc.vector.tensor_tensor(f1[:], sb[:, 0:1024], sb[:, 1024:2048], MIN)
                        f2 = scrp.tile([128, 1024], f16, tag="scr2")
                        nc.vector.tensor_tensor(f2[:], f1[:], stashA[:, b, :], MIN)
                        f2b = scrp.tile([128, 1024], f16, tag="scr2b")
                        nc.vector.tensor_tensor(f2b[:], f2[:], stashB[:, b, :], MIN)
                        f3 = scrp.tile([128, 512], f16, tag="scr3")
                        nc.vector.tensor_tensor(f3[:], f2b[:, 0:512], f2b[:, 512:1024], MIN)
                        f4 = scrp.tile([128, 256], f16, tag="scr4")
                        nc.vector.tensor_tensor(f4[:], f3[:, 0:256], f3[:, 256:512], MIN)
                        nc.vector.tensor_reduce(
                            rmw[:, b:b + 1, g // 4], f4[:], axis=AX, op=MIN
                        )
                cred = scrp.tile([128, GROUP], f16, tag="cred")
                nc.gpsimd.partition_all_reduce(
                    cred[:],
                    R16[:, g * GROUP:(g + 1) * GROUP],
                    channels=128,
                    reduce_op=bass_isa.ReduceOp.max,
                )
                nc.sync.dma_start(
                    colmin_d[:, g * GROUP:(g + 1) * GROUP], cred[0:1, :]
                )

            for b in range(RB):
                nc.vector.tensor_reduce(
                    rowmin_sb[:, b:b + 1], rmw[:, b, :], axis=AX, op=MIN
                )

            nc.sync.dma_start(rowmin_d[:], rowmin_sb[:])
    nc.finalize()
    return nc


def _split16(a32):
    """fp32 [k, n] -> (hi, lo) fp16 pair with hi+lo ~ a32 (22-bit mantissa)."""
    hi = a32.astype(np.float16)
    lo = (a32 - hi.astype(np.float32)).astype(np.float16)
    return hi, lo


def _prep_inputs(set1, set2):
    s1 = np.asarray(set1, dtype=np.float32)
    s2 = np.asarray(set2, dtype=np.float32)
    n1 = (s1.astype(np.float64) ** 2).sum(1)[None].astype(np.float32)
    n2 = (s2.astype(np.float64) ** 2).sum(1)[None].astype(np.float32)
    xh, xl = _split16(s1.T)
    yh, yl = _split16(s2.T)
    nxh, nxl = _split16(n1)
    nyh, nyl = _split16(n2)
    m2yh = (-2.0 * yh.astype(np.float32)).astype(np.float16)  # exact
    m2yl = (-2.0 * yl.astype(np.float32)).astype(np.float16)  # exact
    ones_n = np.ones((1, N), np.float16)
    ones_m = np.ones((1, M), np.float16)
    XA = np.concatenate([xh, xh, xl, nxh, nxl, ones_n, ones_n], axis=0)
    YR = np.concatenate([m2yh, m2yl, m2yh, ones_m, ones_m, nyh, nyl], axis=0)
    assert XA.shape == (KDIM, N) and YR.shape == (KDIM, M)
    return np.ascontiguousarray(XA), np.ascontiguousarray(YR)


def _run(nc, XA, YR, trace=False, **kw):
    in_maps = [
        {"xa": np.ascontiguousarray(XA[:, c * NLOC:(c + 1) * NLOC]), "ya": YR}
        for c in range(NCORES)
    ]
    return run_bass_kernel_spmd(nc, in_maps, list(range(NCORES)), trace=trace, **kw)


def _combine(res):
    rowmins, colmins = [], []
    for i in range(NCORES):
        rowmins.append(res.results[i]["rowmin"].T.ravel())
        colmins.append(res.results[i]["colmin"].ravel())
    rowmin_d2 = -np.concatenate(rowmins).astype(np.float32)
    colmin_d2 = -np.max(np.stack(colmins), axis=0).astype(np.float32)
    term1 = np.sqrt(np.maximum(rowmin_d2, 0.0)).mean()
    term2 = np.sqrt(np.maximum(colmin_d2, 0.0)).mean()
    return np.asarray(term1 + term2, dtype=np.float32)


def kernel(set1: np.ndarray, set2: np.ndarray) -> np.ndarray:
    global _compiled
    if _compiled is None:
        _compiled = _build()
    XA, YR = _prep_inputs(set1, set2)
    res = _run(_compiled, XA, YR)
    return _combine(res)


# revision 17
# speedup vs baseline: 1.0989x; 1.0041x over previous
"""Averaged Hausdorff loss on 8 Trainium2 cores.

Math: d2[i,j] = |x_i|^2 + |y_j|^2 - 2 x_i.y_j via an augmented inner product
on the PE. fp32 matmul runs at 1/4 rate on TRN2, so each fp32 value is split
into hi+lo fp16 halves (~22 effective mantissa bits) and the product expanded
into K=13 fp16 contraction dims (the xl*yl term, ~1e-6, is dropped):
  dims 0-2 : xh_k * (-2 yh_k)
  dims 3-5 : xh_k * (-2 yl_k)
  dims 6-8 : xl_k * (-2 yh_k)
  dims 9-10: |x|^2 (hi, lo) * 1
  dims 11-12: 1 * |y|^2 (hi, lo)
PSUM accumulates in fp32. sqrt is monotonic so mins are taken over d2 and
sqrt'd on the host.

Sharding: set1 rows across 8 cores (2048 rows/core vs all 16384 set2 rows).
Per (128-row block, 2048-col group): 4 matmuls fill a 4-bank PSUM tile; the
Scalar engine evacuates it to SBUF fp16 (also serving as the col-min init on
the first block); the Vector engine then does
  - row-mins: tensor_tensor_reduce on chunk pairs (elementwise min of two
    512-col chunks + free-axis min-reduce chained through rowmin_sb),
  - col-mins: one fp16 2x tensor_tensor min into the running R16 buffer.
Column partials finish with PE transposes + free-axis reduces; the host
min-combines partials across cores, sqrts, and averages.
"""

import numpy as np
from contextlib import ExitStack

import concourse.bacc as bacc
import concourse.mybir as mybir
import concourse.tile as tile
import concourse.bass_isa as bass_isa
from concourse.bass_utils import run_bass_kernel_spmd

f32 = mybir.dt.float32
f16 = mybir.dt.float16
N = 16384
M = 16384
NCORES = 8
NLOC = N // NCORES       # 2048 set1 rows per core
RB = NLOC // 128         # 16 row blocks
CHUNK = 512              # matmul free dim (one PSUM bank of f32)
GROUP = 4 * CHUNK        # 2048 cols per PSUM tile (4 banks)
NG = M // GROUP          # 8 groups
TCOLS = M // 128         # 128 transpose tiles for the column-min finish
KDIM = 13

_compiled = None


def _build():
    nc = bacc.Bacc()
    xa_d = nc.dram_tensor("xa", [KDIM, NLOC], f16, kind="ExternalInput")
    ya_d = nc.dram_tensor("ya", [KDIM, M], f16, kind="ExternalInput")
    rowmin_d = nc.dram_tensor("rowmin", [128, RB], f32, kind="ExternalOutput")
    colmin_d = nc.dram_tensor("colmin", [1, M], f16, kind="ExternalOutput")

    AX = mybir.AxisListType.X
    MIN = mybir.AluOpType.max  # chains run on negated d2

    with tile.TileContext(nc) as tc:
        with ExitStack() as ctx:
            iop = ctx.enter_context(tc.tile_pool(name="io", bufs=1))
            sbp = ctx.enter_context(tc.tile_pool(name="sb16", bufs=4))
            scrp = ctx.enter_context(tc.tile_pool(name="scr", bufs=2))
            psmm = ctx.enter_context(tc.tile_pool(name="psmm", bufs=2, space="PSUM"))

            xa = iop.tile([KDIM, NLOC], f16)
            nc.sync.dma_start(xa[:], xa_d[:])
            ya_t = []
            for g in range(NG):
                yg = iop.tile([KDIM, GROUP], f16, tag=f"ya{g}")
                nc.sync.dma_start(yg[:], ya_d[:, g * GROUP:(g + 1) * GROUP])
                ya_t.append(yg)

            R16 = iop.tile([128, M], f16)        # running col-min, d2, fp16
            rowmin_sb = iop.tile([128, RB], f32)
            f4stash = iop.tile([128, RB, 256], f16)
            stashA = iop.tile([128, RB, 1024], f16)
            stashB = iop.tile([128, RB, 1024], f16)

            for g in range(NG):
                for b in range(RB):
                    ps = psmm.tile([128, GROUP], f32, tag="mm")
                    for k in range(4):
                        c = g * 4 + k
                        nc.tensor.matmul(
                            ps[:, k * CHUNK:(k + 1) * CHUNK],
                            xa[:, b * 128:(b + 1) * 128],
                            ya_t[g][:, k * CHUNK:(k + 1) * CHUNK],
                            start=True,
                            stop=True,
                        )
                    Rg = R16[:, g * GROUP:(g + 1) * GROUP]
                    if b == 0:
                        # evacuate+convert straight into R16 (col-min init)
                        nc.scalar.mul(Rg, ps[:], -1.0)
                        sb = Rg
                    else:
                        sbt = sbp.tile([128, GROUP], f16, tag="sb16")
                        nc.scalar.mul(sbt[:], ps[:], -1.0)
                        sb = sbt[:]
                        nc.vector.tensor_tensor(Rg, Rg, sb, MIN)
                    if g % 2 == 0:
                        # stash this group's 1024-wide fold (A on quads 0, B holds 0+1)
                        nc.vector.tensor_tensor(
                            stashA[:, b, :], sb[:, 0:1024], sb[:, 1024:2048], MIN
                        )
                    elif g % 4 == 1:
                        f1 = scrp.tile([128, 1024], f16, tag="scr")
                        nc.vector.tensor_tensor(f1[:], sb[:, 0:1024], sb[:, 1024:2048], MIN)
                        nc.vector.tensor_tensor(
                            stashB[:, b, :], f1[:], stashA[:, b, :], MIN
                        )
                    else:
                        f1 = scrp.tile([128, 1024], f16, tag="scr")
                        nc.vector.tensor_tensor(f1[:], sb[:, 0:1024], sb[:, 1024:2048], MIN)
                        f2 = scrp.tile([128, 1024], f16, tag="scr2")
                        nc.vector.tensor_tensor(f2[:], f1[:], stashA[:, b, :], MIN)
                        f2b = scrp.tile([128, 1024], f16, tag="scr2b")
                        nc.vector.tensor_tensor(f2b[:], f2[:], stashB[:, b, :], MIN)
                        f3 = scrp.tile([128, 512], f16, tag="scr3")
                        nc.vector.tensor_tensor(f3[:], f2b[:, 0:512], f2b[:, 512:1024], MIN)
                        if g // 4 == 0:
                            # first quad: final fold lands in the stash
                            nc.vector.tensor_tensor(
                                f4stash[:, b, :], f3[:, 0:256], f3[:, 256:512], MIN
                            )
                        else:
                            f4 = scrp.tile([128, 256], f16, tag="scr4")
                            nc.vector.tensor_tensor(f4[:], f3[:, 0:256], f3[:, 256:512], MIN)
                            f5 = scrp.tile([128, 256], f16, tag="scr5")
                            nc.vector.tensor_tensor(f5[:], f4[:], f4stash[:, b, :], MIN)
                            nc.vector.tensor_reduce(
                                rowmin_sb[:, b:b + 1], f5[:], axis=AX, op=MIN
                            )
                cred = scrp.tile([128, GROUP], f16, tag="cred")
                nc.gpsimd.partition_all_reduce(
                    cred[:],
                    R16[:, g * GROUP:(g + 1) * GROUP],
                    channels=128,
                    reduce_op=bass_isa.ReduceOp.max,
                )
                nc.sync.dma_start(
                    colmin_d[:, g * GROUP:(g + 1) * GROUP], cred[0:1, :]
                )

            nc.sync.dma_start(rowmin_d[:], rowmin_sb[:])
    nc.finalize()
    return nc


def _split16(a32):
    """fp32 [k, n] -> (hi, lo) fp16 pair with hi+lo ~ a32 (22-bit mantissa)."""
    hi = a32.astype(np.float16)
    lo = (a32 - hi.astype(np.float32)).astype(np.float16)
    return hi, lo


def _prep_inputs(set1, set2):
    s1 = np.asarray(set1, dtype=np.float32)
    s2 = np.asarray(set2, dtype=np.float32)
    n1 = (s1.astype(np.float64) ** 2).sum(1)[None].astype(np.float32)
    n2 = (s2.astype(np.float64) ** 2).sum(1)[None].astype(np.float32)
    xh, xl = _split16(s1.T)
    yh, yl = _split16(s2.T)
    nxh, nxl = _split16(n1)
    nyh, nyl = _split16(n2)
    m2yh = (-2.0 * yh.astype(np.float32)).astype(np.float16)  # exact
    m2yl = (-2.0 * yl.astype(np.float32)).astype(np.float16)  # exact
    ones_n = np.ones((1, N), np.float16)
    ones_m = np.ones((1, M), np.float16)
    XA = np.concatenate([xh, xh, xl, nxh, nxl, ones_n, ones_n], axis=0)
    YR = np.concatenate([m2yh, m2yl, m2yh, ones_m, ones_m, nyh, nyl], axis=0)
    assert XA.shape == (KDIM, N) and YR.shape == (KDIM, M)
    return np.ascontiguousarray(XA), np.ascontiguousarray(YR)


def _run(nc, XA, YR, trace=False, **kw):
    in_maps = [
        {"xa": np.ascontiguousarray(XA[:, c * NLOC:(c + 1) * NLOC]), "ya": YR}
        for c in range(NCORES)
    ]
    return run_bass_kernel_spmd(nc, in_maps, list(range(NCORES)), trace=trace, **kw)


def _combine(res):
    rowmins, colmins = [], []
    for i in range(NCORES):
        rowmins.append(res.results[i]["rowmin"].T.ravel())
        colmins.append(res.results[i]["colmin"].ravel())
    rowmin_d2 = -np.concatenate(rowmins).astype(np.float32)
    colmin_d2 = -np.max(np.stack(colmins), axis=0).astype(np.float32)
    term1 = np.sqrt(np.maximum(rowmin_d2, 0.0)).mean()
    term2 = np.sqrt(np.maximum(colmin_d2, 0.0)).mean()
    return np.asarray(term1 + term2, dtype=np.float32)


def kernel(set1: np.ndarray, set2: np.ndarray) -> np.ndarray:
    global _compiled
    if _compiled is None:
        _compiled = _build()
    XA, YR = _prep_inputs(set1, set2)
    res = _run(_compiled, XA, YR)
    return _combine(res)
